# revision 7
# baseline (speedup 1.0000x reference)
"""BiLSTM (B=64, L=256, D=512, H=512) on 8 Trainium2 NeuronCores.

Sharding: 8 cores = 2 directions x 4 batch-slices of 16 (weights replicated
per direction, time loop local per core; backward cores get time-reversed x
so all cores run one SPMD program).

v4: two independent batch-8 chains per core, software-pipelined so each
chain's ~3us recurrence latency hides behind the other chain's work.

Per-core program:
  Phase 1 (interleaved): xpart[token, 4H] = x_t @ Wx.T + bias staged in SBUF.
  Per step t, per chain X in {A=batch 0:8, B=8:16}:
    - gates psum P_X (128,512): ident-inject xpart + 16 col-strip matmuls
      (hT_X stationary, Wh moving, 4 strips concurrent);
    - ONE sigmoid over strips f/i/o/g~ (g weights pre-scaled 2x so
      tanh(x)=2*sig(2x)-1);
    - cell update split across engines: u,t2,add on DVE; t1,hmul on GpSimd;
      tanh on ScalarE;
    - hT via XBAR DMA-transpose (h [16,512] -> hT [128,4,16]) off the PE;
    - out_h DMA + xpart prefetch issued from GpSimd (cheap SWDGE issue).
"""

import numpy as np
import ml_dtypes

from concourse import tile, mybir, bacc
from concourse.bass_utils import run_bass_kernel_spmd
from concourse.masks import make_identity

FP = mybir.dt.float32
BF = mybir.dt.bfloat16
AF = mybir.ActivationFunctionType
ALU = mybir.AluOpType

B = 16        # local batch per core
BC = 8        # batch per chain
L = 256       # timesteps
D = 512       # input dim
H = 512       # hidden
NG = 4 * H    # gate width
TOK = L * B   # tokens per core
NM = TOK // 128

_CACHED_NC = None


def _build():
    nc = bacc.Bacc("TRN2", target_bir_lowering=False, debug=False)

    xT = nc.dram_tensor("xT", [D, TOK], BF, kind="ExternalInput").ap()
    W = nc.dram_tensor("W", [D + H, NG], BF, kind="ExternalInput").ap()
    bias = nc.dram_tensor("bias", [1, NG], BF, kind="ExternalInput").ap()
    out_h = nc.dram_tensor("out_h", [L, B, H], BF, kind="ExternalOutput").ap()

    with tile.TileContext(nc, trace_sim=False) as tc:
        with tc.tile_pool(name="wpool", bufs=1) as wpool, \
             tc.tile_pool(name="cpool", bufs=1) as cpool:
            W_t = []
            for k in range(8):
                wt = wpool.tile([128, NG], BF, tag=f"w{k}", name=f"w{k}")
                nc.sync.dma_start(wt[:], W[128 * k:128 * (k + 1), :])
                W_t.append(wt)
            bias_t = wpool.tile([1, NG], BF)
            nc.sync.dma_start(bias_t[:], bias[:, :])
            ones_t = cpool.tile([1, 128], BF)
            nc.vector.memset(ones_t[:, :], 1.0)
            ident = cpool.tile([BC, BC], BF)
            make_identity(nc, ident[:, :])

            with tc.tile_pool(name="p1x", bufs=3) as p1x, \
                 tc.tile_pool(name="xsp", bufs=5) as xsp, \
                 tc.tile_pool(name="p1ps", bufs=2, space="PSUM") as p1ps, \
                 tc.tile_pool(name="xpp", bufs=2) as xpp, \
                 tc.tile_pool(name="st", bufs=2) as st, \
                 tc.tile_pool(name="ch", bufs=2) as ch, \
                 tc.tile_pool(name="gpsA", bufs=2, space="PSUM") as gpsA, \
                 tc.tile_pool(name="gpsB", bufs=2, space="PSUM") as gpsB:
                gps = {0: gpsA, 1: gpsB}

                xps = {}
                xm_map = {}
                p1n = [0]

                def emit_p1_part(m, n):
                    if n == 0:
                        xps[m] = xsp.tile([128, NG], BF, tag="xps",
                                          name=f"xps{m}")
                        xm = p1x.tile([128, 4, 128], BF, tag="xm", name="xm")
                        for k in range(4):
                            nc.sync.dma_start(
                                xm[:, k, :],
                                xT[128 * k:128 * (k + 1),
                                   128 * m:128 * (m + 1)])
                        xm_map[m] = xm
                    xm = xm_map[m]
                    ps = p1ps.tile([128, 512], FP, tag="ps1", name="ps1")
                    for k in range(4):
                        nc.tensor.matmul(
                            ps[:, :], xm[:, k, :],
                            W_t[k][:, 512 * n:512 * (n + 1)],
                            start=(k == 0), stop=False)
                    nc.tensor.matmul(
                        ps[:, :], ones_t[:, :],
                        bias_t[:, 512 * n:512 * (n + 1)],
                        start=False, stop=True)
                    # psum->sbuf stage; alternate ScalarE/DVE to spread load
                    # (GpSimd cannot access PSUM)
                    if p1n[0] % 2 == 0:
                        nc.scalar.copy(
                            xps[m][:, 512 * n:512 * (n + 1)], ps[:, :])
                    else:
                        nc.vector.tensor_copy(
                            xps[m][:, 512 * n:512 * (n + 1)], ps[:, :])
                    p1n[0] += 1

                def emit_p1(m):
                    for n in range(4):
                        emit_p1_part(m, n)

                # per-chain state
                c_prev = []
                hT_prev = []
                xp_t = {}
                for X in range(2):
                    c0 = st.tile([BC, H], BF, tag=f"c{X}", name=f"c{X}_0")
                    nc.vector.memset(c0[:, :], 0.0)
                    c_prev.append(c0)
                    hT0 = st.tile([128, 4, B], BF, tag=f"hT{X}",
                                  name=f"hT{X}_0")
                    nc.vector.memset(hT0[:, :, :], 0.0)
                    hT_prev.append(hT0)

                def emit_xp(t):
                    # per-chain xpart slice staged at partition base 0
                    for X in range(2):
                        xp = xpp.tile([BC, NG], BF, tag=f"xp{X}",
                                      name=f"xp{X}")
                        r0 = B * (t % 8) + BC * X
                        nc.gpsimd.dma_start(
                            xp[:], xps[t // 8][r0:r0 + BC, :])
                        xp_t[(t, X)] = xp

                for m in range(2):
                    emit_p1(m)
                emit_xp(0)
                emit_xp(1)

                for t in range(L):
                    if t % 2 == 0 and t // 8 + 2 < NM:
                        emit_p1_part(t // 8 + 2, (t % 8) // 2)

                    # --- PE: gates for both chains ---
                    P = []
                    for X in range(2):
                        PX = gps[X].tile([128, 512], FP, tag="P",
                                         name=f"P{X}")
                        P.append(PX)
                        xp = xp_t.pop((t, X))
                        for j in range(4):
                            nc.tensor.matmul(
                                PX[32 * j:32 * j + BC, :], ident[:, :],
                                xp[:, 512 * j:512 * (j + 1)],
                                start=True, stop=False,
                                tile_position=(0, 32 * j))
                        for k in range(4):
                            for j in range(4):
                                nc.tensor.matmul(
                                    PX[32 * j:32 * j + BC, :],
                                    hT_prev[X][:, k, 0:BC],
                                    W_t[4 + k][:, 512 * j:512 * (j + 1)],
                                    start=False, stop=(k == 3),
                                    tile_position=(0, 32 * j))

                    # --- ScalarE: one sigmoid per chain over f/i/o/g~ ---
                    s = []
                    for X in range(2):
                        sX = ch.tile([112, H], BF, tag=f"s{X}", name=f"s{X}")
                        nc.scalar.activation(sX[:, :], P[X][0:112, :],
                                             AF.Sigmoid)
                        s.append(sX)

                    # --- cell update, spread DVE/Pool ---
                    # u placed at rows 32:40 so the t2 tensor_tensor's two
                    # inputs share a start partition (walrus requirement)
                    u, t1, t2 = [], [], []
                    for X in range(2):
                        uX = ch.tile([32 + BC, H], BF, tag=f"u{X}",
                                     name=f"u{X}")
                        nc.vector.tensor_scalar(
                            uX[32:32 + BC, :], s[X][96:96 + BC, :], 2.0, -1.0,
                            op0=ALU.mult, op1=ALU.add)
                        u.append(uX)
                    for X in range(2):
                        t1X = ch.tile([BC, H], BF, tag=f"t1{X}",
                                      name=f"t1{X}")
                        nc.gpsimd.tensor_mul(t1X[:, :], s[X][0:BC, :],
                                             c_prev[X][:, :])
                        t1.append(t1X)
                    for X in range(2):
                        t2X = ch.tile([BC, H], BF, tag=f"t2{X}",
                                      name=f"t2{X}")
                        nc.vector.tensor_mul(t2X[:, :], s[X][32:32 + BC, :],
                                             u[X][32:32 + BC, :])
                        t2.append(t2X)
                    c_new = []
                    for X in range(2):
                        cX = st.tile([BC, H], BF, tag=f"c{X}", name=f"c{X}")
                        nc.vector.tensor_add(cX[:, :], t1[X][:, :],
                                             t2[X][:, :])
                        c_new.append(cX)
                    # th at rows 64:72 to pair with the o strip for hmul
                    th = []
                    for X in range(2):
                        thX = ch.tile([64 + BC, H], BF, tag=f"th{X}",
                                      name=f"th{X}")
                        nc.scalar.activation(thX[64:64 + BC, :],
                                             c_new[X][:, :], AF.Tanh)
                        th.append(thX)
                    h = []
                    for X in range(2):
                        hX = st.tile([B, H], BF, tag=f"h{X}", name=f"h{X}")
                        nc.gpsimd.tensor_mul(hX[0:BC, :], s[X][64:64 + BC, :],
                                             th[X][64:64 + BC, :])
                        h.append(hX)

                    # --- hT via XBAR dma transpose; out_h; xp prefetch ---
                    hT_new = []
                    for X in range(2):
                        hTX = st.tile([128, 4, B], BF, tag=f"hT{X}",
                                      name=f"hT{X}")
                        nc.sync.dma_start(hTX[:, :, :], h[X][:, :],
                                          transpose=True)
                        hT_new.append(hTX)
                    for X in range(2):
                        nc.gpsimd.dma_start(
                            out_h[t, BC * X:BC * X + BC, :], h[X][0:BC, :])
                    if t + 2 < L:
                        emit_xp(t + 2)

                    c_prev = c_new
                    hT_prev = hT_new
    nc.compile()
    return nc


def _host_prepare(x_full, weights, direction, bslice):
    xs = x_full[bslice]
    if direction == "bw":
        xs = xs[:, ::-1, :]
    xT = np.ascontiguousarray(xs.transpose(2, 1, 0).reshape(D, TOK))
    Wc = np.concatenate(
        [weights[f"W_{direction}_{n}"].T for n in "fiog"], axis=1).copy()
    bc = np.concatenate(
        [weights[f"b_{direction}_{n}"] for n in "fiog"])[None, :].copy()
    # tanh fold: g strip pre-activations scaled by 2 (tanh(x) = 2*sig(2x)-1)
    Wc[:, 3 * H:] *= 2.0
    bc[:, 3 * H:] *= 2.0
    return {"xT": np.ascontiguousarray(xT).astype(ml_dtypes.bfloat16),
            "W": np.ascontiguousarray(Wc).astype(ml_dtypes.bfloat16),
            "bias": np.ascontiguousarray(bc).astype(ml_dtypes.bfloat16)}


def kernel(**inputs):
    global _CACHED_NC
    inputs = {k: np.asarray(v) for k, v in inputs.items()}
    x = inputs["x"]
    Bx, Lx, _ = x.shape
    assert (Bx, Lx) == (64, L)

    if _CACHED_NC is None:
        _CACHED_NC = _build()
    nc = _CACHED_NC

    in_maps = []
    meta = []
    for ci in range(8):
        d = "fw" if ci < 4 else "bw"
        bs = (ci % 4) * B
        in_maps.append(_host_prepare(x, inputs, d, slice(bs, bs + B)))
        meta.append((d, bs))

    res = run_bass_kernel_spmd(nc, in_maps, core_ids=list(range(8)))

    hf = np.zeros((L, Bx, H), np.float32)
    hb = np.zeros((L, Bx, H), np.float32)
    for ci in range(8):
        d, bs = meta[ci]
        oh = np.asarray(res.results[ci]["out_h"]).astype(np.float32)
        if d == "fw":
            hf[:, bs:bs + B, :] = oh
        else:
            hb[:, bs:bs + B, :] = oh[::-1]

    # faithful to the reference: stack time-major, flatten, hstack, reshape
    flat = np.concatenate([hf.reshape(-1, H), hb.reshape(-1, H)], axis=1)
    return flat.reshape(Bx, Lx, 2 * H).astype(np.float32)
